# revision 2
# baseline (speedup 1.0000x reference)
"""Trainium2 Bass kernel for nn_BandwidthPredictorNNHall — v3.

v2 scheme (host staging, symmetric upper-triangle, stationary-K Z stage)
plus:
  - bias-free exp: the act bias -r_i/2 rides as a 5th contraction row of
    the Gram matmul (lhsT rows [q, nh_bf16], rhs rows [q, 1]); the bf16
    rounding of nh is exactly cancelled by pre-scaling the device-side Mq
    rows with e^{-eps} on the host.
  - with no per-row bias, row tiles merge into 5 activation instructions
    ({512},{640+384},{768+256},{896+128},{1024}) instead of 8, saving
    3 SBUF-access bubbles and ~600ns of ScalarE time.
  - the three input DMAs issue from three different engines' DGE queues
    (SP/Act/DVE) so their 625ns HWDGE slots run in parallel.

Device per row tile group: Gram chunks (bank-split) -> one Exp -> per
(row, col>=row) tiny Z matmul with the K'' block stationary, accumulating
Z_c = sum_{r<c} B_rc^T Mq'_r + 0.5 B_cc Mq'_c into one PSUM bank
(pre-zeroed by a single start=True matmul). Host: unscale rows by
e^{-r_j/2}, W' = Z^T Mq, V = W' + W'^T, then the reference's scalar tail.
"""

import sys

sys.path.insert(0, "/opt/trn_rl_repo")

import numpy as np

try:
    import ml_dtypes

    _BF16 = ml_dtypes.bfloat16
except Exception:  # pragma: no cover
    _BF16 = None

_B, _N, _D = 8, 1024, 4
_P = 128
_NT = _N // _P
_NM = 1 + 2 * _D
_INV_SQRT_2PI = 1.0 / np.sqrt(2.0 * np.pi)
_RK = 0.282095
_FACT = 1.0592 * float(_N) ** (-1.0 / (4 + _D))

# act groups: rows per group, chosen so each group is <=1024 cols and the
# first group (smallest Gram) gates the stream start
_GROUPS = [(4,), (3, 7), (2, 6), (1, 5), (0,)]
_N_BIASED = 1  # first group is single-row with a real act bias


def _row_width(ir):
    return (_NT - ir) * _P


def _group_layout(g):
    """-> list of (ir, offset) and total width for act group g."""
    rows, off = [], 0
    for ir in _GROUPS[g]:
        rows.append((ir, off))
        off += _row_width(ir)
    return rows, off


_NC = None


def _build_kernel():
    import concourse.bass as bass  # noqa: F401
    import concourse.tile as tile
    from concourse import bacc, mybir

    f32 = mybir.dt.float32
    bf16 = mybir.dt.bfloat16
    Act = mybir.ActivationFunctionType

    nc = bacc.Bacc("TRN2", target_bir_lowering=False, debug=False, num_devices=_B)
    # rows 0-3: q (d-major); row 4: nh (paired against the ones row of
    # qR in the 5-row Gram); row 32: nh again, at an aligned base so the
    # bias transposes may read it
    qB_in = nc.dram_tensor("qb", [33, _NT, _P], bf16, kind="ExternalInput")
    mb_in = nc.dram_tensor("mb", [_P, _NT, 2 * _NM], bf16, kind="ExternalInput")
    z_out = nc.dram_tensor("zout", [_P, _NT, _NM], bf16, kind="ExternalOutput")

    with tile.TileContext(nc) as tc:
        with (
            tc.tile_pool(name="singles", bufs=1) as singles,
            tc.tile_pool(name="psG", bufs=2, space="PSUM") as psG,
            tc.tile_pool(name="psP", bufs=1, space="PSUM") as psPp,
            tc.tile_pool(name="psB", bufs=1, space="PSUM") as psB,
        ):
            qB = singles.tile([33, _NT, _P], bf16, tag="qb")
            nc.sync.dma_start(out=qB, in_=qB_in[:])
            qL = qB[0 : _D + 1]
            ones33 = singles.tile([33, 1], bf16, tag="ones33")
            nc.gpsimd.memset(ones33, 1.0)
            zbias = singles.tile([_P, 1], f32, tag="zbias")
            nc.gpsimd.memset(zbias, 0.0)
            mball = singles.tile([_P, _NT, 2 * _NM], bf16, tag="mb")
            nc.gpsimd.dma_start(out=mball, in_=mb_in[:])
            # rhs variant: same q rows, ones in the bias row (memset early,
            # q rows copied once the DMA lands)
            qR = singles.tile([_D + 1, _NT, _P], bf16, tag="qr")
            nc.vector.memset(qR[:], 1.0)
            nc.vector.tensor_copy(qR[0:_D], qL[0:_D])

            # warm the Exp table during the DMA wait
            warm = singles.tile([1, 1], f32, tag="warm")
            nc.vector.memset(warm, 0.0)
            warm2 = singles.tile([1, 1], f32, tag="warm2")
            nc.scalar.activation(out=warm2, in_=warm, func=Act.Exp, bias=zbias[0:1])

            qRf = qR[:].rearrange("d c i -> d (c i)")  # [5, 1024] view
            KT = singles.tile([_P, len(_GROUPS), 1280], bf16, tag="kt")
            psP = psPp.tile([_P, _NT, _NM], f32, tag="psp")
            # pre-zero psP via DVE during the DMA wait; the Z matmuls all
            # accumulate with start=False onto it (no PE start flag ever
            # touches the bank, and no early PE work resets the p-state
            # ramp reference, keeping the first Gram at full clock)
            nc.vector.memset(psP[:], 0.0)


            psg_t = {}

            qLf4 = qL[0:_D].rearrange("d c i -> d (c i)")

            # biases for the single-row groups: nh row (partition 32) ->
            # [128,1] per-partition columns via tiny outer-product matmuls,
            # then a scalar-engine Copy each to SBUF. Emitted between
            # gram(0) and act(0) so the first Gram keeps the full-clock
            # p-state (no earlier PE instruction resets the ramp reference).
            nhsb = singles.tile([_P, _N_BIASED], f32, tag="nhsb")

            def bias_setup():
                for k in range(_N_BIASED):
                    ir = _GROUPS[k][0]
                    psb = psB.tile([_P, 1], f32, tag=f"psb{k}")
                    nc.tensor.matmul(
                        psb,
                        lhsT=qB[32:33, ir, :],
                        rhs=ones33[32:33],
                        start=True,
                        stop=True,
                    )
                    nc.scalar.activation(
                        out=nhsb[:, k : k + 1], in_=psb, func=Act.Copy
                    )

            def gram(g):
                rows, wg = _group_layout(g)
                psg = psG.tile([_P, 1280], f32, tag="psg")
                psg_t[g] = psg
                for ir, off in rows:
                    w = _row_width(ir)
                    # chunk so no matmul output crosses a 512-f32 PSUM bank
                    c0 = off
                    while c0 < off + w:
                        c1 = min(off + w, (c0 // 512 + 1) * 512)
                        src = ir * _P + (c0 - off)
                        if g < _N_BIASED:
                            # single-row group: nh rides as the act bias, so
                            # this Gram needs neither the nh row nor qR --
                            # it can fire the moment the qL DMA lands
                            nc.tensor.matmul(
                                psg[:, c0:c1],
                                lhsT=qL[0:_D, ir, :],
                                rhs=qLf4[:, src : src + (c1 - c0)],
                                start=True,
                                stop=True,
                            )
                        else:
                            nc.tensor.matmul(
                                psg[:, c0:c1],
                                lhsT=qL[:, ir, :],
                                rhs=qRf[:, src : src + (c1 - c0)],
                                start=True,
                                stop=True,
                            )
                        c0 = c1

            def act(g):
                rows, wg = _group_layout(g)
                if g < _N_BIASED:
                    nc.scalar.activation(
                        out=KT[:, g, 0:wg],
                        in_=psg_t[g][:, 0:wg],
                        func=Act.Exp,
                        bias=nhsb[:, g : g + 1],
                    )
                else:
                    nc.scalar.activation(
                        out=KT[:, g, 0:wg],
                        in_=psg_t[g][:, 0:wg],
                        func=Act.Exp,
                        bias=zbias,
                    )

            def zmm(g, last):
                rows, _ = _group_layout(g)
                for ir, off in rows:
                    for c in range(ir, _NT):
                        koff = off + (c - ir) * _P
                        rhs = (
                            mball[:, ir, _NM : 2 * _NM]
                            if c == ir
                            else mball[:, ir, 0:_NM]
                        )
                        nc.tensor.matmul(
                            psP[:, c, :],
                            lhsT=KT[:, g, koff : koff + _P],
                            rhs=rhs,
                            start=False,
                            stop=(last and ir == rows[-1][0] and c == _NT - 1),
                            skip_group_check=True,
                        )

            ng = len(_GROUPS)
            bias_setup()
            gram(0)
            act(0)
            for g in range(1, ng):
                gram(g)
                zmm(g - 1, last=False)
                act(g)
            zmm(ng - 1, last=True)

            zsb = singles.tile([_P, _NT, _NM], bf16, tag="zsb")
            nc.vector.tensor_copy(zsb, psP)
            nc.sync.dma_start(out=z_out[:], in_=zsb)

    nc.compile()
    return nc


def _get_nc():
    global _NC
    if _NC is None:
        _NC = _build_kernel()
    return _NC


def _stage(pb):
    """Host staging for one batch: particles [1024, 4] f32 -> device inputs."""
    std = pb.astype(np.float64).std(axis=0, ddof=1)
    pilot = _FACT * std
    q = (pb / pilot.astype(np.float32)[None, :]).astype(np.float32)
    qv = q.astype(_BF16).astype(np.float32)  # bf16-rounded ground truth
    r = (qv.astype(np.float64) ** 2).sum(1)
    nhb = (-0.5 * r).astype(np.float32).astype(_BF16)  # bias actually applied
    eps = nhb.astype(np.float64) + 0.5 * r  # bias rounding error
    Mq = np.concatenate([np.ones((_N, 1), np.float32), qv, qv * qv], 1)
    Mqd = (Mq.astype(np.float64) * np.exp(-eps)[:, None]).astype(np.float32)

    q_ict = qv.reshape(_P, _NT, _D)
    qT = q_ict.transpose(2, 1, 0)  # [4, 8, 128]
    nhT = nhb.astype(np.float32).reshape(_P, _NT).T.reshape(1, _NT, _P)
    pad = np.zeros((27, _NT, _P), np.float32)
    qBc = np.concatenate([qT, nhT, pad, nhT], 0).astype(_BF16)
    mb = np.concatenate([Mqd, 0.5 * Mqd], 1).reshape(_P, _NT, 2 * _NM).astype(_BF16)
    return (
        {"qb": np.ascontiguousarray(qBc), "mb": np.ascontiguousarray(mb)},
        pilot,
        r,
        Mq,
    )


def _finalize(zout, pilot, r, Mq):
    Z = np.asarray(zout).astype(np.float64).reshape(_N, _NM)
    Zc = Z * np.exp(-0.5 * r)[:, None]
    W = Zc.T @ Mq.astype(np.float64)
    V = W + W.T
    d = np.arange(_D)
    s2 = (
        V[0, 5 + d] + V[5 + d, 0] - 2.0 * V[1 + d, 1 + d] - V[0, 0]
    ) * _INV_SQRT_2PI
    denom = _N * (_N - 1)
    I2 = s2 / pilot**5 / denom
    base = _RK / I2 / _N
    return (np.sign(base) * np.abs(base) ** 0.2).astype(np.float32)


def kernel(particles, weights=None, **_unused):
    from concourse.bass_utils import run_bass_kernel_spmd

    particles = np.ascontiguousarray(np.asarray(particles), dtype=np.float32)
    assert particles.shape == (_B, _N, _D), particles.shape

    nc = _get_nc()
    staged = [_stage(particles[c]) for c in range(_B)]
    in_maps = [s[0] for s in staged]
    res = run_bass_kernel_spmd(nc, in_maps, list(range(_B)))

    out = np.empty((_B, _D), np.float32)
    for c in range(_B):
        _, pilot, r, Mq = staged[c]
        out[c] = _finalize(np.asarray(res.results[c]["zout"]), pilot, r, Mq)
    return out
